# revision 1
# baseline (speedup 1.0000x reference)
"""Trainium2 Bass kernel for a causal self-attention block with LoRA adapters.

Model (B=2, T=2048, C=1024, H=16 heads, hd=64, LoRA r=32, scale 0.5):
    qkv = x @ w_attn.T + b_attn + 0.5*(x @ la_attn.T) @ lb_attn.T
    y   = causal_softmax_attention(q, k, v)
    out = y @ w_proj.T + b_proj + 0.5*(y @ la_proj.T) @ lb_proj.T

Sharding: Megatron-style tensor parallel over 8 NeuronCores. Each core owns
2 heads: column-split c_attn (its q/k/v rows), full attention for its heads,
row-split c_proj producing a partial-sum output; the host sums the 8 partials
(the "all-reduce") and transposes back.

Device algorithm per core (all matmuls bf16 with fp32 PSUM accumulation):
  - fold LoRA into effective weights on-device: W_eff = W + 0.5 * lb @ la
  - x.T resident in SBUF as bf16 [C, B*T] (host passes x.T, SWDGE casts)
  - qT/kT = W_qk_eff @ x.T   -> [256, 4096]  (channels on partitions)
  - v natural = x @ W_v_eff  -> [4096, 128]  (tokens on partitions)
  - per (batch, head, 1024-wide q chunk): S.T[k, q] = kT.T @ qT blocks into
    2-bank PSUM tiles; P = exp(S/8) on ScalarE (no max subtraction; |S| < 3
    for this distribution; one ACT instruction per k-tile); causal mask
    applied in-place on the diagonal 128x128 block only; [v | 1].T @ P
    accumulated over k tiles gives numerator rows 0..63 and the softmax
    denominator in row 64; 1/denom = exp(-ln(denom)) on the partition-64
    ScalarE lane (the combined Ln+Exp ACT table is pre-loaded once);
    broadcast across partitions with a K=1 matmul; multiply.
  - outT_partial = W_proj_eff.T @ y_norm.T per head (K=64), + b_proj/8.
    Projection chunks for batch 0 are emitted interleaved into batch 1's
    attention so they fill PE/DVE gaps while ScalarE crunches exp.
Output: bf16 partial [C, B*T] per core; host sums in fp32.
"""

from contextlib import ExitStack

import numpy as np
import ml_dtypes

import concourse.bass as bass
import concourse.tile as tile
from concourse import bacc, mybir
from concourse.bass_utils import run_bass_kernel_spmd

F32 = mybir.dt.float32
BF16 = mybir.dt.bfloat16
AF = mybir.ActivationFunctionType
ALU = mybir.AluOpType

B, T, C, H, R = 2, 2048, 1024, 16, 32
HD = C // H              # 64
NCORES = 8
HPC = H // NCORES        # 2 heads per core
CH = HPC * HD            # 128 per-core channels
BT = B * T               # 4096
NCT = C // 128           # 8 contraction tiles
NR = 3 * CH              # 384 qkv rows per core
KT = T // 128            # 16 key tiles per sequence
QCH = 512                # q chunk size
NQC = T // QCH           # 4 q chunks per sequence
TCH = 512                # token chunk for qkv/proj
NTC = BT // TCH          # 8

_CACHE: dict = {}
_PHASE_MARKS: list = []
_ABLATE: set = set()


def _mark(nc, name):
    _PHASE_MARKS.append((name, nc.next_id()))


def _emit(ctx: ExitStack, tc: tile.TileContext, t_in: dict, outT, reps: int = 1):
    nc = tc.nc
    _PHASE_MARKS.clear()
    _mark(nc, "setup")

    singles = ctx.enter_context(tc.tile_pool(name="singles", bufs=1))
    wst = ctx.enter_context(tc.tile_pool(name="wst", bufs=4))
    psA = ctx.enter_context(tc.tile_pool(name="psA", bufs=2, space=bass.MemorySpace.PSUM))
    psST = ctx.enter_context(tc.tile_pool(name="psST", bufs=2, space=bass.MemorySpace.PSUM))
    psY = ctx.enter_context(tc.tile_pool(name="psY", bufs=1, space=bass.MemorySpace.PSUM))
    ptp = ctx.enter_context(tc.tile_pool(name="ptp", bufs=12))
    yup = ctx.enter_context(tc.tile_pool(name="yup", bufs=3))
    dnp = ctx.enter_context(tc.tile_pool(name="dnp", bufs=2))
    outp = ctx.enter_context(tc.tile_pool(name="outp", bufs=10))

    # ---------- constants / weights to SBUF ----------
    la_sb = singles.tile([R, C], F32)
    nc.sync.dma_start(la_sb[:], t_in["la_attn"][:])
    lbq_sb = singles.tile([R, NR], F32)
    nc.sync.dma_start(lbq_sb[:], t_in["lbqkvT"][:])
    lapc_sb = singles.tile([R, HPC, HD], F32)
    nc.sync.dma_start(lapc_sb[:], t_in["lapc"][:])
    lbp_sb = singles.tile([R, C], F32)
    nc.sync.dma_start(lbp_sb[:], t_in["lbpT"][:])
    bq_sb = singles.tile([128, 3], F32)
    nc.sync.dma_start(bq_sb[:], t_in["b_qkv"][:].rearrange("(m p) -> p m", p=128))
    bp_sb = singles.tile([128, NCT], F32)
    nc.sync.dma_start(bp_sb[:], t_in["bp8"][:].rearrange("(m p) -> p m", p=128))
    bv_sb = singles.tile([1, CH], F32)
    nc.sync.dma_start(bv_sb[:], t_in["bv_row"][:])
    mask_sb = singles.tile([128, 128], BF16)
    nc.sync.dma_start(mask_sb[:], t_in["masks"][:])
    ones_t = singles.tile([128, 128], F32)
    nc.vector.memset(ones_t[:], 1.0)

    # Pre-load the ACT table set that serves BOTH Exp and Ln, so the
    # act-table-load pass never needs to thrash between per-func sets.
    nc.scalar.add_instruction(mybir.InstLoadActFuncSet(
        name=nc.get_next_instruction_name(), act_func_set_id=6, ins=[], outs=[]))

    # ---------- x.T -> SBUF bf16 (cast in DMA) ----------
    _mark(nc, "xload")
    xb = singles.tile([128, NCT, BT], BF16)
    xT = t_in["xT"]
    if "xload" not in _ABLATE:
        # first two 512-col half-blocks gate qk0's first groups; split them
        # so the earliest data lands sooner
        for sl in (slice(0, 512), slice(512, 1024)):
            for ct in range(NCT):
                nc.gpsimd.dma_start(xb[:, ct, sl],
                                    xT[ct * 128:(ct + 1) * 128, sl])
        for q4 in range(1, 4):
            for ct in range(NCT):
                sl = slice(q4 * (BT // 4), (q4 + 1) * (BT // 4))
                nc.gpsimd.dma_start(xb[:, ct, sl],
                                    xT[ct * 128:(ct + 1) * 128, sl])

    # ---------- fold LoRA into effective weights ----------
    _mark(nc, "fold")
    la_b = singles.tile([R, C], BF16)
    nc.vector.tensor_copy(la_b[:], la_sb[:])
    lbq_b = singles.tile([R, NR], BF16)
    nc.vector.tensor_copy(lbq_b[:], lbq_sb[:])
    lapc_b = singles.tile([R, HPC, HD], BF16)
    nc.vector.tensor_copy(lapc_b[:], lapc_sb[:])
    lbp_b = singles.tile([R, C], BF16)
    nc.vector.tensor_copy(lbp_b[:], lbp_sb[:])
    wq_eff = singles.tile([128, NCT, NR], BF16)
    for ct in range(NCT):
        w_raw = wst.tile([128, NR], F32, tag="wq_raw")
        nc.sync.dma_start(
            w_raw[:], t_in["wqkvT"][ct * 128:(ct + 1) * 128, :])
        f = psA.tile([128, NR], F32, tag="a")
        nc.tensor.matmul(f[:], la_b[:, ct * 128:(ct + 1) * 128], lbq_b[:],
                         start=True, stop=True)
        nc.vector.scalar_tensor_tensor(
            wq_eff[:, ct, :], f[:], 0.5, w_raw[:], ALU.mult, ALU.add)

    # proj: [128, C] effective weight; head h's d-rows live at partitions
    # h*64..h*64+63 (fold matmul writes PSUM at that base directly)
    wp_eff = singles.tile([128, C], BF16)
    w_raw = singles.tile([128, C], F32, name="wp_raw")
    nc.sync.dma_start(w_raw[:], t_in["wpT"][:])
    for h in range(HPC):
        hs = slice(h * HD, (h + 1) * HD)
        for half in range(2):
            f = psA.tile([128, 512], F32, tag="a", name=f"fp{h}_{half}")
            nc.tensor.matmul(f[hs, :], lapc_b[:, h, :],
                             lbp_b[:, half * 512:(half + 1) * 512],
                             start=True, stop=True)
            nc.vector.scalar_tensor_tensor(
                wp_eff[hs, half * 512:(half + 1) * 512], f[hs, :], 0.5,
                w_raw[hs, half * 512:(half + 1) * 512], ALU.mult, ALU.add)

    # v bias broadcast across partitions: [128, CH]
    bvb_ps = psA.tile([128, CH], F32, tag="a")
    nc.tensor.matmul(bvb_ps[:], ones_t[0:1, :], bv_sb[:], start=True, stop=True)
    bvb = singles.tile([128, CH], F32)
    nc.vector.tensor_copy(bvb[:], bvb_ps[:])

    for _rep in range(reps):
        # ---------- qk/v for both batches, attention, then projections ----------
        qkT = singles.tile([128, 2, BT], BF16)
        v_ext = singles.tile([128, B, HPC, KT, HD + 1], BF16)
        nc.vector.memset(v_ext[:, :, :, :, HD:HD + 1], 1.0)
        yn = singles.tile([128, BT], BF16)  # normalized y.T, ch on partitions
        if "attn" in _ABLATE:
            nc.vector.memset(yn[:], 1.0)

        def emit_qkv(b):
            _mark(nc, f"qk{b}")
            for qc in range(b * NQC, (b + 1) * NQC):
                for mt in range(2):
                    sl = slice(qc * TCH, (qc + 1) * TCH)
                    ps = psA.tile([128, TCH], F32, tag="a",
                                  name=f"qk{b}_{mt}_{qc}")
                    for ct in range(NCT):
                        nc.tensor.matmul(
                            ps[:], wq_eff[:, ct, mt * 128:(mt + 1) * 128],
                            xb[:, ct, sl], start=(ct == 0),
                            stop=(ct == NCT - 1))
                    nc.vector.tensor_scalar(qkT[:, mt, sl], ps[:],
                                            bq_sb[:, mt:mt + 1], None,
                                            ALU.add)
            _mark(nc, f"v{b}")
            for kt in range(KT):
                tt = b * KT + kt
                ps = psA.tile([128, CH], F32, tag="a", name=f"v{b}_{kt}")
                for ct in range(NCT):
                    nc.tensor.matmul(
                        ps[:], xb[:, ct, tt * 128:(tt + 1) * 128],
                        wq_eff[:, ct, 2 * CH:3 * CH],
                        start=(ct == 0), stop=(ct == NCT - 1))
                for h in range(HPC):
                    nc.vector.tensor_tensor(
                        v_ext[:, b, h, kt, 0:HD], ps[:, h * HD:(h + 1) * HD],
                        bvb[:, h * HD:(h + 1) * HD], ALU.add)

        def emit_attn(b):
            _mark(nc, f"attn{b}")
            QW = 1024
            for h in range(HPC):
                hp = slice(h * HD, (h + 1) * HD)
                for j2 in range(T // QW):
                    yp = psY.tile([HD + 1, QW], F32, tag="y",
                                  name=f"yp{b}_{h}_{j2}")
                    q0 = b * T + j2 * QW
                    for kt in range(8 * j2 + 8):
                        lead = (kt // 8 == j2)
                        cs = 128 * (kt % 8) if lead else 0
                        k_lhs = qkT[hp, 1,
                                    b * T + kt * 128: b * T + (kt + 1) * 128]
                        st = psST.tile([128, QW], F32, tag="st")
                        for lo, hi in (((cs, 512), (512, QW)) if cs < 512
                                       else ((cs, QW),)):
                            nc.tensor.matmul(st[:, lo:hi], k_lhs,
                                             qkT[hp, 0, q0 + lo: q0 + hi],
                                             start=True, stop=True)
                        pt = ptp.tile([128, QW], BF16, tag="pt")
                        nc.scalar.activation(pt[:, cs:], st[:, cs:], AF.Exp,
                                             scale=0.125)
                        if lead:
                            # causal mask on the diagonal 128x128 block
                            nc.vector.tensor_tensor(
                                pt[:, cs:cs + 128], pt[:, cs:cs + 128],
                                mask_sb[:], ALU.mult)
                        if cs < 512:
                            nc.tensor.matmul(
                                yp[:, cs:512], v_ext[:, b, h, kt, :],
                                pt[:, cs:512], start=(kt == 0),
                                stop=(kt == 8 * j2 + 3))
                        lo = max(cs, 512)
                        nc.tensor.matmul(
                            yp[:, lo:QW], v_ext[:, b, h, kt, :],
                            pt[:, lo:QW], start=(kt == 0),
                            stop=(kt == 8 * j2 + 7))
                    # stage numerator+denominator to SBUF, free PSUM
                    yu = yup.tile([HD + 1, QW], F32, tag="yu")
                    nc.vector.tensor_copy(yu[:], yp[:])
                    # 1/denom = exp(-ln(denom)) on the partition-64 ACT lane
                    rc = dnp.tile([HD + 1, 2, QW], F32, tag="rc")
                    nc.scalar.activation(rc[HD:HD + 1, 0, :],
                                         yu[HD:HD + 1, :], AF.Ln)
                    nc.scalar.activation(rc[HD:HD + 1, 1, :],
                                         rc[HD:HD + 1, 0, :], AF.Exp,
                                         scale=-1.0)
                    # broadcast 1/denom across 64 partitions via K=1 matmuls
                    for lo in (0, 512):
                        dbt = psA.tile([HD, 512], F32, tag="a",
                                       name=f"db{b}_{h}_{j2}_{lo}")
                        nc.tensor.matmul(dbt[:],
                                         ones_t[HD:HD + 1, 0:HD],
                                         rc[HD:HD + 1, 1, lo:lo + 512],
                                         start=True, stop=True)
                        if h == 0:
                            nc.vector.tensor_tensor(
                                yn[0:HD, q0 + lo: q0 + lo + 512],
                                yu[0:HD, lo:lo + 512], dbt[:], ALU.mult)
                        else:
                            # h1 rows belong at partitions 64..127; engines
                            # are lane-tied, so relocate via SBUF->SBUF DMA
                            ynt = dnp.tile([HD, 512], BF16, tag="ynt")
                            nc.vector.tensor_tensor(
                                ynt[:], yu[0:HD, lo:lo + 512], dbt[:],
                                ALU.mult)
                            nc.sync.dma_start(
                                yn[HD:128, q0 + lo: q0 + lo + 512], ynt[:])
                    if h == 1:
                        for mt in range(NCT):
                            for tc8 in (b * NQC + 2 * j2,
                                        b * NQC + 2 * j2 + 1):
                                _PROJ_QUEUE.append((b, mt, tc8))
                    _emit_proj_chunks(tc, nc, psA, outp, wp_eff, yn,
                                      bp_sb, outT, _PROJ_QUEUE,
                                      (16 if (b == 1 and h == 0) else 12),
                                      tail=(b == 1 and h == 1 and j2 == 1))

        emit_qkv(0)
        emit_qkv(1)
        if "attn" not in _ABLATE:
            with tc.high_priority(offset=310):
                emit_attn(0)
            emit_attn(1)

        if "attn" in _ABLATE:
            for mt in range(NCT):
                for tc8 in range(2 * NQC):
                    _PROJ_QUEUE.append((tc8 // NQC, mt, tc8))
        _mark(nc, "projtail")
        if "proj" in _ABLATE:
            _PROJ_QUEUE.clear()
        while _PROJ_QUEUE:
            _emit_proj_chunks(tc, nc, psA, outp, wp_eff, yn, bp_sb, outT,
                              _PROJ_QUEUE, len(_PROJ_QUEUE), tail=True)


_PROJ_QUEUE: list = []


def _emit_proj_chunks(tc, nc, psA, outp, wp_eff, yn, bp_sb, outT, queue, n,
                      tail=False):
    # process tc8 pairs: two 512-col chunks share one [128, 1024] store
    save = tc.cur_priority
    tc.cur_priority = save + 8000   # gap-fill only; don't preempt attention
    try:
        for _ in range(min(n, len(queue)) // 2):
            b, mt, tc8 = queue.pop(0)
            b2, mt2, tc8b = queue.pop(0)
            assert mt2 == mt and tc8b == tc8 + 1
            ot = outp.tile([128, 2, TCH], BF16, tag="ot")
            for half, t8 in enumerate((tc8, tc8b)):
                sl = slice(t8 * TCH, (t8 + 1) * TCH)
                po = psA.tile([128, TCH], F32, tag="a",
                              name=f"po{b}_{mt}_{t8}")
                nc.tensor.matmul(po[:], wp_eff[:, mt * 128:(mt + 1) * 128],
                                 yn[:, sl], start=True, stop=True)
                if not tail or (mt + half) % 2 == 0:
                    nc.vector.tensor_scalar(ot[:, half, :], po[:],
                                            bp_sb[:, mt:mt + 1], None,
                                            ALU.add)
                else:
                    nc.scalar.activation(ot[:, half, :], po[:], AF.Identity,
                                         bias=bp_sb[:, mt:mt + 1])
            nc.sync.dma_start(
                outT[mt * 128:(mt + 1) * 128, tc8 * TCH:(tc8 + 2) * TCH],
                ot[:])
    finally:
        tc.cur_priority = save


def _declare_io(nc):
    t_in = {
        "xT": nc.dram_tensor("xT", [C, BT],
                             BF16 if "xbf16" in _ABLATE else F32,
                             kind="ExternalInput"),
        "wqkvT": nc.dram_tensor("wqkvT", [C, NR], F32, kind="ExternalInput"),
        "lbqkvT": nc.dram_tensor("lbqkvT", [R, NR], F32, kind="ExternalInput"),
        "la_attn": nc.dram_tensor("la_attn", [R, C], F32, kind="ExternalInput"),
        "b_qkv": nc.dram_tensor("b_qkv", [NR], F32, kind="ExternalInput"),
        "wpT": nc.dram_tensor("wpT", [CH, C], F32, kind="ExternalInput"),
        "lapc": nc.dram_tensor("lapc", [R, HPC, HD], F32, kind="ExternalInput"),
        "lbpT": nc.dram_tensor("lbpT", [R, C], F32, kind="ExternalInput"),
        "bp8": nc.dram_tensor("bp8", [C], F32, kind="ExternalInput"),
        "bv_row": nc.dram_tensor("bv_row", [1, CH], F32, kind="ExternalInput"),
        "masks": nc.dram_tensor("masks", [128, 128], BF16, kind="ExternalInput"),
    }
    outT = nc.dram_tensor("outT", [C, BT], BF16, kind="ExternalOutput")
    return t_in, outT


def _build(reps: int = 1):
    nc = bacc.Bacc("TRN2", target_bir_lowering=False, debug=False)
    t_in, outT = _declare_io(nc)
    with tile.TileContext(nc) as tc:
        with ExitStack() as ctx:
            _emit(ctx, tc, t_in, outT, reps=reps)
    nc.compile()
    return nc


def _make_in_maps(inputs: dict) -> list:
    f32 = np.float32
    x = np.asarray(inputs["x"], f32).reshape(BT, C)
    w_attn = np.asarray(inputs["w_attn"], f32)
    b_attn = np.asarray(inputs["b_attn"], f32)
    la_attn = np.ascontiguousarray(np.asarray(inputs["la_attn"], f32))
    lb_attn = np.asarray(inputs["lb_attn"], f32)
    w_proj = np.asarray(inputs["w_proj"], f32)
    b_proj = np.asarray(inputs["b_proj"], f32)
    la_proj = np.asarray(inputs["la_proj"], f32)
    lb_proj = np.asarray(inputs["lb_proj"], f32)

    xT = np.ascontiguousarray(x.T)                       # [C, BT]
    if "xbf16" in _ABLATE:
        xT = xT.astype(ml_dtypes.bfloat16)
    lbpT = np.ascontiguousarray(lb_proj.T)               # [R, C]

    # triangular causal tile: M[k, q] = 1 if q >= k
    k_idx = np.arange(128)[:, None]
    q_idx = np.arange(128)[None, :]
    masks = (q_idx >= k_idx).astype(ml_dtypes.bfloat16)

    in_maps = []
    for core in range(NCORES):
        ch0 = core * CH
        rows = np.r_[ch0:ch0 + CH, C + ch0:C + ch0 + CH,
                     2 * C + ch0:2 * C + ch0 + CH]
        lapc = np.ascontiguousarray(
            la_proj[:, ch0:ch0 + CH].reshape(R, HPC, HD))
        in_maps.append({
            "xT": xT,
            "wqkvT": np.ascontiguousarray(w_attn[rows].T),
            "lbqkvT": np.ascontiguousarray(lb_attn[rows].T),
            "la_attn": la_attn,
            "b_qkv": np.ascontiguousarray(b_attn[rows]),
            "wpT": np.ascontiguousarray(w_proj[:, ch0:ch0 + CH].T),
            "lapc": lapc,
            "lbpT": lbpT,
            "bp8": np.ascontiguousarray(b_proj / NCORES),
            "bv_row": np.ascontiguousarray(b_attn[2 * C + ch0:2 * C + ch0 + CH]
                                           .reshape(1, CH)),
            "masks": masks,
        })
    return in_maps


def _execute(inputs: dict, trace: bool = False):
    if "nc" not in _CACHE:
        _CACHE["nc"] = _build()
    nc = _CACHE["nc"]
    in_maps = _make_in_maps(inputs)
    res = run_bass_kernel_spmd(nc, in_maps, core_ids=list(range(NCORES)),
                               trace=trace)
    acc = np.zeros((C, BT), np.float32)
    for r in res.results:
        acc += np.asarray(r["outT"], dtype=np.float32)
    out = np.ascontiguousarray(acc.T).reshape(B, T, C).astype(np.float32)
    return out, res


def kernel(**inputs) -> np.ndarray:
    out, _ = _execute(inputs, trace=False)
    return out

